# revision 22
# baseline (speedup 1.0000x reference)
"""Trainium2 Bass kernel for batched single-head attention + output projection + layernorm.

Reference computation (per batch element b):
    q = Q@Wq + bq ; k = K@Wk + bk ; v = V@Wv + bv
    S = q k^T / sqrt(DV) ; S[pad_mask==0] = -1e9 ; P = softmax(S)
    out = LN(P v @ Wo + bo; g0, beta0)

Sharding: data-parallel over batch B=8 across the 8 NeuronCores (one batch
element per core, no collectives).

Math folds (exact up to bf16 rounding paths):
  - masked keys contribute exactly 0 (exp underflows), so K/V are compacted
    on the host to the kept keys (padded to a 128 multiple, pad bias -1e5).
    With Bernoulli(0.5) masks this halves all key-dimension matmul work.
  - both weight products fold onto the COMPACTED K/V side (NK_c ~ 1152 <
    NQ = 2048), which is the cheaper association:
      S   = Q (Wq Wk^T) K_raw^T = Q @ K''      with K'' = M K_raw^T  (phase A)
      out = P V_raw (Wv Wo)     = P @ V'       with V'  = V_raw Wvo  (phase B)
    Host precomputes M^T and Wvo (weight-weight products only); the
    input-dependent K''/V' projections run on device each iteration.
    This costs 2 x NK_c*DQ*DV instead of 2 x NQ*DQ*DV (the old q-side
    projection + separate out-projection), a ~20% MAC reduction, and the
    P @ V' matmul directly yields the output in natural [token, feature]
    layout (lhsT = E^T slab slices), so layernorm reads the psum in place
    and there are still zero on-device transposes.
  - bk drops out (q . bk is constant per query row -> softmax invariant);
    bq folds into the per-key exp bias: scale*(bq Wk^T) . K_j, host-applied;
    bv and bo fold into boe = bv@Wo + bo (softmax rows sum to 1).
  - softmax normalization is deferred: O_unnorm = E@V' with E = exp(S'),
    and in the lean case (all biases zero, trivial LN affine - true for
    this problem) dropped entirely: LN is invariant to per-row scale.
  - pad mask + 1/sqrt(DV) scale fuse into the Exp activation:
    E^T = exp(S^T * scale + mbias[j]).

Layout strategy: host passes Q^T/K^T/V^T (bf16) so every contraction dim
lands on SBUF partitions; attention runs in transposed score layout
(S^T[j,i]) so the pad mask is a per-partition bias, and the final P @ V'
contraction (over keys) emits natural [token, feature] psum for LN.

Schedule engineering:
  - all loop-invariant-shaped operands (M^T, K^T, V^T, Wvo, the it=0
    q block, mb) live in "hot" tiles refilled mid-body right after their
    last use, so the first matmuls after a loop back-edge never wait on
    DMA completion latency and every phase transition has its inputs
    resident.
  - phase D interleaves the two psum feature-halves per E^T weight load
    (1024 streamed columns per LDWEIGHTS), phase B likewise; a
    post-compile pass (dedup_ldweights) deletes the redundant
    back-to-back LDWEIGHTS those pairs produce.
  - single 8-bank PSUM pool; LN reads the phase-D psums directly (no
    copy-out stage); output stores spread across gpsimd/scalar/sync DMA
    queues so the back-edge drains overlap across engines.

Measured HW model (slope-timed For_i body, 8-core SPMD):
    body_ns ~= matmul_rows / 1.94GHz + ~13ns * n_matmul
The effective PE clock is ~1.94GHz (not 2.4) because the chip power-
throttles when all 8 cores stream bf16 matmuls continuously (the same
body measures ~2.26GHz effective on a single core), so this kernel sits
at the power-capped roofline; NK must stay padded to 128 multiples
(partial-partition tail chunks cost more in PE tile_size reconfigs than
their saved rows).
"""

import numpy as np
import ml_dtypes

import concourse.bass as bass
import concourse.bacc as bacc
import concourse.tile as tile
from concourse import mybir
from concourse.bass_utils import run_bass_kernel_spmd

BF16 = mybir.dt.bfloat16
F32 = mybir.dt.float32
AF = mybir.ActivationFunctionType
P = 128
N_CORES = 8
EPS = 1e-5

# Full-problem shapes (hardcoded; the grading harness runs kernel() standalone).
B, NQ, NK, DQ, DV = 8, 2048, 2048, 1024, 1024


def hot_alloc(tc, pool, ins, blk=512):
    """Resident tiles refilled mid-body each iteration: the folded weights
    (M^T, Wvo), the raw K^T/V^T inputs, the it=0 q block, and the exp bias."""
    qt = ins["qt"]
    DQ_, NQ_ = qt.shape
    NK_ = ins["kt"].shape[1]
    DV_ = ins["wvo"].shape[1]
    C = DQ_ // P
    D = DV_ // P
    JS = NK_ // P
    IW = min(blk, NQ_)
    return {
        "mt": pool.tile([P, C, DQ_], BF16, tag="mthot", bufs=1, name="mt_hot"),
        "kt": [pool.tile([P, NK_], BF16, tag="kthot", bufs=C, name=f"kt_hot{c}")
               for c in range(C)],
        "vt": [pool.tile([P, NK_], BF16, tag="vthot", bufs=C, name=f"vt_hot{c}")
               for c in range(C)],
        "wvo": pool.tile([P, D, DV_], BF16, tag="wvohot", bufs=1, name="wvo_hot"),
        "qin": [pool.tile([P, IW], BF16, tag="qinhot", bufs=C, name=f"qin_hot{c}")
                for c in range(C)],
        "mb": pool.tile([P, JS], F32, tag="mbhot", bufs=1, name="mb_hot"),
        "iw": IW,
    }


def _refill_mt_kt(nc, hot, ins):
    mt, kt = ins["mt"], ins["kt"]
    C = mt.shape[0] // P
    for b in range(C):
        nc.gpsimd.dma_start(out=hot["mt"][:, b, :], in_=mt[b * P:(b + 1) * P, :])
        nc.gpsimd.dma_start(out=hot["kt"][b], in_=kt[b * P:(b + 1) * P, :])


def _refill_vt_wvo(nc, hot, ins):
    vt, wvo = ins["vt"], ins["wvo"]
    C = vt.shape[0] // P
    D = wvo.shape[0] // P
    for d in range(C):
        nc.sync.dma_start(out=hot["vt"][d], in_=vt[d * P:(d + 1) * P, :])
    for d in range(D):
        nc.sync.dma_start(out=hot["wvo"][:, d, :], in_=wvo[d * P:(d + 1) * P, :])


def _refill_qin(nc, hot, ins):
    qt = ins["qt"]
    C = qt.shape[0] // P
    IW = hot["iw"]
    for c in range(C):
        nc.gpsimd.dma_start(out=hot["qin"][c], in_=qt[c * P:(c + 1) * P, 0:IW])


def _refill_mb(nc, hot, ins):
    nc.gpsimd.dma_start(out=hot["mb"], in_=ins["mb"].rearrange("(j p) -> p j", p=P))


def hot_load(tc, hot, ins):
    nc = tc.nc
    _refill_mt_kt(nc, hot, ins)
    _refill_vt_wvo(nc, hot, ins)
    _refill_qin(nc, hot, ins)
    _refill_mb(nc, hot, ins)


def attention_body(tc, outs, ins, blk=512, lean=True, hot=None):
    nc = tc.nc
    qt = ins["qt"]
    out = outs["out"]

    DQ_, NQ_ = qt.shape
    NK_ = ins["kt"].shape[1]
    DV_ = ins["wvo"].shape[1]
    C = DQ_ // P          # contraction 128-chunks of the projections
    D = DV_ // P          # projected-feature 128-chunks
    JS = NK_ // P         # key 128-chunks
    IW = min(blk, NQ_)    # query block width (psum free dim)
    EW = min(blk, DV_)    # feature block width
    NI = NQ_ // IW        # query blocks
    NE = DV_ // EW        # feature blocks
    IS = IW // P          # query 128-chunks per query block
    scale = float(DV_) ** -0.5

    mt_sb = hot["mt"]
    kt_sb = hot["kt"]
    vt_sb = hot["vt"]
    wvo_sb = hot["wvo"]
    mb_sb = hot["mb"]

    # key-dim blocks of <=512 for the phase-A psum tiles
    jblks = []
    j0 = 0
    while j0 < NK_:
        jw = min(512, NK_ - j0)
        jblks.append((j0, jw))
        j0 += jw

    with tc.tile_pool(name="sb", bufs=1) as sb, \
         tc.tile_pool(name="ps", bufs=8, space="PSUM") as ps:

        # ---------------- constants ----------------
        eps_sb = sb.tile([P, 1], F32, tag="eps", bufs=2, name="eps_sb")
        nc.vector.memset(eps_sb, EPS)
        if not lean:
            ones_col = sb.tile([P, 1], BF16, tag="ones", bufs=2, name="ones_col")
            nc.vector.memset(ones_col, 1.0)

            boe, g0, b0 = ins["boe"], ins["g0"], ins["b0"]

            def bcast(ap, nm):
                t = sb.tile([P, DV_], F32, tag=nm, bufs=1, name=nm)
                nc.gpsimd.dma_start(
                    out=t,
                    in_=bass.AP(tensor=ap.tensor, offset=ap.offset,
                                ap=[[0, P]] + [list(a) for a in ap.ap]),
                )
                return t

            boe_b = bcast(boe, "boe_b")
            g0_b = bcast(g0, "g0_b")
            b0_b = bcast(b0, "b0_b")

        qin_tiles = {}

        def issue_qin(it):
            lst = []
            for c in range(C):
                t = sb.tile([P, IW], BF16, tag="qin", bufs=2 * C,
                            name=f"qin{it}_{c}")
                nc.sync.dma_start(out=t, in_=qt[c * P:(c + 1) * P,
                                               it * IW:(it + 1) * IW])
                lst.append(t)
            qin_tiles[it] = lst

        if NI > 1:
            issue_qin(1)

        # ---------------- phase A: K'' = M K_raw^T ----------------
        # kq_sb[a][p, j] = sum_b M[a*P+p, b] K^T[b, j]  (lhsT = M^T chunks)
        kq_sb = [sb.tile([P, NK_], BF16, tag="kq", bufs=C, name=f"kq_sb{a}")
                 for a in range(C)]
        for a in range(C):
            for (j0, jw) in jblks:
                pp = ps.tile([P, 512], F32, tag="ps", name=f"ppa{a}_{j0}")
                for b in range(C):
                    nc.tensor.matmul(pp[:, :jw],
                                     mt_sb[:, b, a * P:(a + 1) * P],
                                     kt_sb[b][:, j0:j0 + jw],
                                     start=(b == 0), stop=(b == C - 1))
                nc.scalar.activation(out=kq_sb[a][:, j0:j0 + jw],
                                     in_=pp[:, :jw], func=AF.Copy)
        _refill_mt_kt(nc, hot, ins)

        # ---------------- phase B: V' = V_raw Wvo ----------------
        # v_sb[j][p, e] = sum_d V^T[d, j*P+p] Wvo[d, e]  (lhsT = V^T slices)
        v_sb = [sb.tile([P, DV_], BF16, tag="v", bufs=JS, name=f"v_sb{j}")
                for j in range(JS)]
        for j in range(JS):
            pps = [ps.tile([P, EW], F32, tag="ps", name=f"ppb{j}_{e}")
                   for e in range(NE)]
            for d in range(C):
                for e in range(NE):
                    nc.tensor.matmul(pps[e],
                                     vt_sb[d][:, j * P:(j + 1) * P],
                                     wvo_sb[:, d, e * EW:(e + 1) * EW],
                                     start=(d == 0), stop=(d == C - 1))
            for e in range(NE):
                nc.vector.tensor_copy(v_sb[j][:, e * EW:(e + 1) * EW], pps[e])
        _refill_vt_wvo(nc, hot, ins)

        # ---------------- phase C/D: per query block ----------------
        for it in range(NI):
            qin = hot["qin"] if it == 0 else qin_tiles.pop(it)
            if it == 1:
                _refill_qin(nc, hot, ins)

            # scores^T + exp (mask & scale fused): et[j] = [128(key), IW] bf16
            et = []
            for j in range(JS):
                pp = ps.tile([P, IW], F32, tag="ps", name=f"pps{it}_{j}")
                for a in range(C):
                    nc.tensor.matmul(pp, kq_sb[a][:, j * P:(j + 1) * P],
                                     qin[a], start=(a == 0), stop=(a == C - 1))
                e_t = sb.tile([P, IW], BF16, tag="et", bufs=JS, name=f"et{it}_{j}")
                nc.scalar.activation(out=e_t, in_=pp, func=AF.Exp, scale=scale,
                                     bias=mb_sb[:, j:j + 1])
                et.append(e_t)
            if it + 2 < NI:
                issue_qin(it + 2)
            if it == NI - 1:
                _refill_mb(nc, hot, ins)

            # attention output in natural layout: psum[s][128(query), EW]
            # = sum_j E^T[j-chunk, s-slab]^T V'[j-chunk, e-block].
            # In the lean case the softmax denominator is skipped entirely:
            # it scales each output row uniformly and layernorm is invariant
            # to per-row scale (no bias between attention and LN).
            for s in range(IS):
                pps = [ps.tile([P, EW], F32, tag="ps",
                               name=f"ppy{it}_{s}_{e}") for e in range(NE)]
                if lean:
                    for j in range(JS):
                        for e in range(NE):
                            nc.tensor.matmul(pps[e],
                                             et[j][:, s * P:(s + 1) * P],
                                             v_sb[j][:, e * EW:(e + 1) * EW],
                                             start=(j == 0), stop=(j == JS - 1))
                else:
                    dpp = ps.tile([P, 512], F32, tag="ps", name=f"ppd{it}_{s}")
                    for j in range(JS):
                        for e in range(NE):
                            nc.tensor.matmul(pps[e],
                                             et[j][:, s * P:(s + 1) * P],
                                             v_sb[j][:, e * EW:(e + 1) * EW],
                                             start=(j == 0), stop=(j == JS - 1))
                        nc.tensor.matmul(dpp[:, 0:1],
                                         et[j][:, s * P:(s + 1) * P], ones_col,
                                         start=(j == 0), stop=(j == JS - 1))

                ysb = sb.tile([P, DV_], F32, tag="y", bufs=3, name=f"y{it}_{s}")
                stats = sb.tile([P, NE, 6], F32, tag="st", bufs=4, name=f"st{it}_{s}")
                if lean:
                    # stats read the psums directly; -mu computes while
                    # sqrt/recip run; the normalize IDENTITY also reads the
                    # psums, so there is no separate copy-out stage at all.
                    for e in range(NE):
                        nc.vector.bn_stats(out=stats[:, e, :], in_=pps[e])
                    mv = sb.tile([P, 2], F32, tag="mv", bufs=4, name=f"mv{it}_{s}")
                    nc.vector.bn_aggr(out=mv, in_=stats)
                    nmu = sb.tile([P, 1], F32, tag="nmu", bufs=4, name=f"nmu{it}_{s}")
                    nc.vector.tensor_scalar_mul(nmu, mv[:, 0:1], -1.0)
                    std = sb.tile([P, 1], F32, tag="std", bufs=4, name=f"std{it}_{s}")
                    nc.scalar.activation(out=std, in_=mv[:, 1:2], func=AF.Sqrt,
                                         bias=eps_sb)
                    rstd = sb.tile([P, 1], F32, tag="rstd", bufs=4, name=f"rstd{it}_{s}")
                    nc.vector.reciprocal(rstd, std)
                    nmr = sb.tile([P, 1], F32, tag="nmr", bufs=4, name=f"nmr{it}_{s}")
                    nc.vector.tensor_mul(nmr, nmu, rstd)
                    for e in range(NE):
                        nc.scalar.activation(out=ysb[:, e * EW:(e + 1) * EW],
                                             in_=pps[e], func=AF.Identity,
                                             scale=rstd, bias=nmr)
                else:
                    recip = sb.tile([P, 1], F32, tag="recip", bufs=4,
                                    name=f"recip{it}_{s}")
                    nc.vector.reciprocal(recip, dpp[:, 0:1])
                    for e in range(NE):
                        nc.scalar.activation(out=ysb[:, e * EW:(e + 1) * EW],
                                             in_=pps[e], func=AF.Identity,
                                             scale=recip)
                    nc.vector.tensor_add(ysb, ysb, boe_b)
                    for e in range(NE):
                        nc.vector.bn_stats(out=stats[:, e, :],
                                           in_=ysb[:, e * EW:(e + 1) * EW])
                    mv = sb.tile([P, 2], F32, tag="mv", bufs=4, name=f"mv{it}_{s}")
                    nc.vector.bn_aggr(out=mv, in_=stats)
                    std = sb.tile([P, 1], F32, tag="std", bufs=4, name=f"std{it}_{s}")
                    nc.scalar.activation(out=std, in_=mv[:, 1:2], func=AF.Sqrt,
                                         bias=eps_sb)
                    rstd = sb.tile([P, 1], F32, tag="rstd", bufs=4, name=f"rstd{it}_{s}")
                    nc.vector.reciprocal(rstd, std)
                    nmr = sb.tile([P, 1], F32, tag="nmr", bufs=4, name=f"nmr{it}_{s}")
                    nc.vector.tensor_mul(nmr, mv[:, 0:1], rstd)
                    nc.vector.tensor_scalar_mul(nmr, nmr, -1.0)
                    nc.scalar.activation(out=ysb, in_=ysb, func=AF.Identity,
                                         scale=rstd, bias=nmr)
                    nc.vector.tensor_mul(ysb, ysb, g0_b)
                    nc.vector.tensor_add(ysb, ysb, b0_b)
                r0 = it * IW + s * P
                # spread stores across DMA queues; the final slabs store via
                # sync/scalar so back-edge DMA drains overlap across engines
                if it == NI - 1 and s == IS - 1:
                    nc.scalar.dma_start(out=out[r0:r0 + P, :], in_=ysb)
                elif it == NI - 1 and s == IS - 2:
                    nc.sync.dma_start(out=out[r0:r0 + P, :], in_=ysb)
                elif s % 2 == 0:
                    nc.gpsimd.dma_start(out=out[r0:r0 + P, :], in_=ysb)
                else:
                    nc.scalar.dma_start(out=out[r0:r0 + P, :], in_=ysb)


def _ld_sig(inst):
    """Identity of an InstLdweights' loaded weights (None = not comparable)."""
    ap = inst.ins[0]
    if getattr(ap, "dynamic_ap_info", None) is not None:
        return None
    return (str(ap.memsetref), str(ap.memref), int(ap.offset), str(ap.ap),
            str(ap.dtype), str(getattr(inst, "tile_size", None)),
            str(getattr(inst, "tile_position", None)),
            str(getattr(inst, "perf_mode", None)),
            str(getattr(inst, "is_transpose", None)))


def dedup_ldweights(nc):
    """Remove back-to-back redundant LDWEIGHTS on the PE stream.

    bass lowers each matmul to LDWEIGHTS + MATMULT(ldweights=False); when
    consecutive matmuls share the stationary operand the later loads are
    no-ops (the array already holds those weights; intervening non-self-
    loading MATMULTs don't clobber them). Only wait/update-free LDWEIGHTS
    are deleted, so semaphore counts are untouched. State resets at block
    boundaries and on any other PE instruction."""
    PE = mybir.EngineType.PE
    removed = 0
    for fn in nc.m.functions:
        for blk in fn.blocks:
            insts = blk.instructions
            last_sig = None
            to_del = []
            for idx in range(len(insts)):
                inst = insts[idx]
                if getattr(inst, "engine", None) != PE:
                    continue
                nm = type(inst).__name__
                if nm == "InstLdweights":
                    si = inst.sync_info
                    clean = si is None or (len(si.on_wait) == 0
                                           and len(si.on_update) == 0)
                    sig = _ld_sig(inst)
                    if clean and sig is not None and sig == last_sig:
                        to_del.append(idx)
                    else:
                        last_sig = sig
                elif nm == "InstMatmult":
                    if getattr(inst, "ldweights", False):
                        last_sig = None
                else:
                    last_sig = None
            for idx in reversed(to_del):
                del insts[idx]
            removed += len(to_del)
    return removed


def build_nc(nq=NQ, nk=NK, dq=DQ, dv=DV, repeat=1, blk=512, hw_loop=0, lean=True):
    nc = bacc.Bacc("TRN2", target_bir_lowering=False, debug=False)
    ins = {
        "qt": nc.dram_tensor("qt", [dq, nq], BF16, kind="ExternalInput").ap(),
        "kt": nc.dram_tensor("kt", [dq, nk], BF16, kind="ExternalInput").ap(),
        "vt": nc.dram_tensor("vt", [dv, nk], BF16, kind="ExternalInput").ap(),
        "mb": nc.dram_tensor("mb", [nk], F32, kind="ExternalInput").ap(),
        "mt": nc.dram_tensor("mt", [dq, dq], BF16, kind="ExternalInput").ap(),
        "wvo": nc.dram_tensor("wvo", [dq, dv], BF16, kind="ExternalInput").ap(),
    }
    if not lean:
        ins.update({
            "boe": nc.dram_tensor("boe", [dv], F32, kind="ExternalInput").ap(),
            "g0": nc.dram_tensor("g0", [dv], F32, kind="ExternalInput").ap(),
            "b0": nc.dram_tensor("b0", [dv], F32, kind="ExternalInput").ap(),
        })
    outs = {"out": nc.dram_tensor("out", [nq, dv], F32, kind="ExternalOutput").ap()}
    with tile.TileContext(nc) as tc:
        with tc.tile_pool(name="hot", bufs=1) as hotp:
            hot = hot_alloc(tc, hotp, ins, blk=blk)
            hot_load(tc, hot, ins)
            if hw_loop:
                # staggered_reset avoids the default all-engine barrier in
                # the loop's per-iteration semaphore reset block, so the PE
                # never idles waiting for the LN/store tail at the back edge
                with tc.For_i(0, hw_loop, 1,
                              staggered_reset=True,
                              hint_engines=(mybir.EngineType.PE,
                                            mybir.EngineType.Activation,
                                            mybir.EngineType.DVE,
                                            mybir.EngineType.SP)):
                    attention_body(tc, outs, ins, blk=blk, lean=lean, hot=hot)
            else:
                for _ in range(repeat):
                    attention_body(tc, outs, ins, blk=blk, lean=lean, hot=hot)
    nc.compile()
    dedup_ldweights(nc)
    return nc


_NC_CACHE = {}


def detect_lean(Wo, bq, bk, bv, bo, g0, beta0):
    f32 = np.float32
    boe = np.asarray(bv, f32) @ np.asarray(Wo, f32) + np.asarray(bo, f32)
    return bool(
        np.all(np.asarray(bq) == 0)
        and np.all(boe == 0)
        and np.all(np.asarray(g0) == 1)
        and np.all(np.asarray(beta0) == 0)
    )


def detect_nk_c(pad_mask):
    """Padded key count after host-side compaction: masked keys contribute
    exactly 0 to softmax numerator and denominator (exp underflows to 0),
    so only kept keys are shipped, padded to a 128 multiple."""
    pm = np.asarray(pad_mask)
    eff = int((pm[:, 0, :] != 0).sum(axis=1).max())
    return max((eff + P - 1) // P * P, P)


def make_in_maps(Q, K, V, pad_mask, Wq, bq, Wk, bk, Wv, bv, Wo, bo, g0, beta0,
                 lean=None, nk=None):
    bf16 = ml_dtypes.bfloat16
    f32 = np.float32
    if lean is None:
        lean = detect_lean(Wo, bq, bk, bv, bo, g0, beta0)
    if nk is None:
        nk = detect_nk_c(pad_mask)
    Q, K, V = np.asarray(Q, f32), np.asarray(K, f32), np.asarray(V, f32)
    pad_mask = np.asarray(pad_mask)
    Wq, Wk, Wv, Wo = (np.asarray(w, f32) for w in (Wq, Wk, Wv, Wo))
    bq, bv, bo = np.asarray(bq, f32), np.asarray(bv, f32), np.asarray(bo, f32)
    g0, beta0 = np.asarray(g0, f32), np.asarray(beta0, f32)

    scale = f32(1.0 / np.sqrt(Wq.shape[1]))
    M = (Wq @ Wk.T).astype(f32)          # folded score weight  [DQ, DK]
    Wvo = (Wv @ Wo).astype(f32)          # folded v-side weight [DK, DV]
    bqk = (bq @ Wk.T).astype(f32)        # per-key score bias from bq

    shared = {"mt": np.ascontiguousarray(M.T).astype(bf16),
              "wvo": Wvo.astype(bf16)}
    if not lean:
        shared.update({
            "boe": (bv @ Wo + bo).astype(f32), "g0": g0, "b0": beta0,
        })
    in_maps = []
    for b in range(Q.shape[0]):
        m = dict(shared)
        m["qt"] = Q[b].T.astype(bf16)
        keep = np.nonzero(pad_mask[b, 0] != 0)[0]
        kc = np.zeros((nk, K.shape[2]), f32)
        vc = np.zeros((nk, V.shape[2]), f32)
        kc[:len(keep)] = K[b][keep]
        vc[:len(keep)] = V[b][keep]
        mbv = np.full(nk, -1e5, f32)
        # bq contributes scale*(bq Wk^T).K_j to each kept key's score bias
        mbv[:len(keep)] = scale * (kc[:len(keep)] @ bqk)
        m["kt"] = kc.T.astype(bf16)
        m["vt"] = vc.T.astype(bf16)
        m["mb"] = mbv
        in_maps.append(m)
    return in_maps


def kernel(Q, K, V, pad_mask, Wq, bq, Wk, bk, Wv, bv, Wo, bo, g0, beta0):
    lean = detect_lean(Wo, bq, bk, bv, bo, g0, beta0)
    nk_c = detect_nk_c(pad_mask)
    key = ("nc", lean, nk_c)
    if key not in _NC_CACHE:
        _NC_CACHE[key] = build_nc(nk=nk_c, lean=lean)
    nc = _NC_CACHE[key]
    in_maps = make_in_maps(Q, K, V, pad_mask, Wq, bq, Wk, bk, Wv, bv, Wo, bo,
                           g0, beta0, lean=lean, nk=nk_c)
    res = run_bass_kernel_spmd(nc, in_maps, core_ids=list(range(N_CORES)))
    return np.stack([res.results[c]["out"] for c in range(N_CORES)], axis=0)
